# revision 1
# baseline (speedup 1.0000x reference)
"""Trainium2 distributed kernel for AnatomicalConsistencyLoss.

Sharding: 8 cores = (batch b in {0,1}) x (depth quarter q in {0..3}).
Each core processes 40 output D-planes (full H,W) of one batch element.

Device layout: each core's shard is pre-blocked on the host into 128
partition-blocks, each a 22x22x22 (d,h,w) bf16 subvolume = a 20^3 output
block plus a 1-voxel halo on every side (zeros at global volume edges).
All three separable 3-tap Sobel passes then become contiguous 1-D
shifted tensor_tensor ops on VectorE with even element offsets (DVE 2x
bf16 mode); ScalarE handles squares/ln/exp and the fused per-partition
accumulations; GpSimd is deliberately idle (its SBUF port locks out
VectorE).  The loss is decomposed as
  mag = sum(s_p) + sum(s_t) - 2*sum(sqrt(s_p*s_t)),
  cos = sum(dot * exp(-0.5*ln(s_p*s_t))),
so sqrt/rsqrt share one Ln and all reductions ride ScalarE accum_out.

Per-core output: [128, 16] fp32 partial sums (4 d-slabs x {sum s_p,
sum s_t, sum sqrt(q), sum cos}) which the host reduces to the scalar.
"""

import sys

import numpy as np

sys.path.insert(0, "/opt/trn_rl_repo")

import ml_dtypes

N_CORES = 8
BC = 20          # block core size
BB = 22          # block size with halo
FB = BB * BB * BB
NVOX = 2 * 160 * 160 * 160
WEIGHT = 0.2
EPS_MAG = 1e-8

_cache = {}


def _build():
    import concourse.bacc as bacc
    import concourse.bass as bass
    import concourse.tile as tile
    from concourse import mybir

    f32 = mybir.dt.float32
    bf16 = mybir.dt.bfloat16
    AF = mybir.ActivationFunctionType
    ALU = mybir.AluOpType

    nc = bacc.Bacc(
        "TRN2",
        target_bir_lowering=False,
        debug=False,
        enable_asserts=False,
        num_devices=N_CORES,
    )
    xp_d = nc.dram_tensor("pred", [128, FB], bf16, kind="ExternalInput")
    xt_d = nc.dram_tensor("targ", [128, FB], bf16, kind="ExternalInput")
    out_d = nc.dram_tensor("out", [128, 16], f32, kind="ExternalOutput")

    NSLAB = 4
    SD = BC // NSLAB       # output planes per slab (5)
    SDH = SD + 2           # input planes per slab (7)
    PL = BB * BB           # 484
    FS = SDH * PL          # 3388: slab flat size, [7, 22, 22] layout
    HS = BB                # h shift = 22
    DS = PL                # d shift = 484
    W1 = FS - 2            # 3386: W-pass extent
    H1 = W1 - HS           # 3364
    H2 = H1 - HS           # 3342
    D1 = H2 - DS           # 2858
    D2 = D1 - DS           # 2374: field valid extent
    FF = SD * PL           # 2420: field tile size
    NVV = 2000             # valid voxels per slab per block

    with tile.TileContext(nc) as tc:
        with tc.tile_pool(name="pers", bufs=1) as pers, \
             tc.tile_pool(name="work", bufs=1) as work, \
             tc.tile_pool(name="fld", bufs=2) as fld:
            accs = pers.tile([128, 4 * NSLAB], f32, tag="accs")

            xs = {}
            cuts = [0, 7 * PL, 12 * PL, 17 * PL, FB]
            for name, dram in (("p", xp_d), ("t", xt_d)):
                x = pers.tile([128, FB], bf16, tag=f"x_{name}")
                for ci in range(4):
                    nc.sync.dma_start(out=x[:, cuts[ci]:cuts[ci + 1]],
                                      in_=dram[:, cuts[ci]:cuts[ci + 1]])
                xs[name] = x

            def conv_fields(name, s):
                """Separable Sobel, one input, one d-slab, all on VectorE.

                Flat 1-D contiguous bf16 ops with even element offsets so
                every op hits a DVE fast mode (GpSimd is kept idle: its
                SBUF access locks out the DVE port pair).  Tensors keep
                junk lanes at block h/w edges ([*,22,22] blocks).
                Returns gx,gy,gz [128, FF] tiles, valid flat [0:D2].
                """
                xf = xs[name][:, SD * PL * s:SD * PL * s + FS]
                t = work.tile([128, FS], bf16, tag="t")
                nc.vector.tensor_add(t[:, 0:W1], xf[:, 0:W1], xf[:, 2:FS])
                u2 = work.tile([128, FS], bf16, tag="u2")
                nc.scalar.activation(u2[:, 0:W1], xf[:, 1:FS - 1], AF.Identity,
                                     scale=2.0)
                sw = work.tile([128, FS], bf16, tag="sw")
                nc.vector.tensor_add(sw[:, 0:W1], t[:, 0:W1], u2[:, 0:W1])
                dw = work.tile([128, FS], bf16, tag="t")
                nc.vector.tensor_sub(dw[:, 0:W1], xf[:, 2:FS], xf[:, 0:W1])

                uh1 = work.tile([128, FS], bf16, tag="uh1")
                nc.vector.tensor_add(uh1[:, 0:H1], sw[:, 0:H1], sw[:, HS:W1])
                shsw = work.tile([128, FS], bf16, tag="shsw")
                dhsw = work.tile([128, FS], bf16, tag="dhsw")
                nc.vector.tensor_add(shsw[:, 0:H2], uh1[:, 0:H2], uh1[:, HS:H1])
                nc.vector.tensor_sub(dhsw[:, 0:H2], uh1[:, HS:H1], uh1[:, 0:H2])
                uh2 = work.tile([128, FS], bf16, tag="uh1")
                nc.vector.tensor_add(uh2[:, 0:H1], dw[:, 0:H1], dw[:, HS:W1])
                shdw = work.tile([128, FS], bf16, tag="sw")
                nc.vector.tensor_add(shdw[:, 0:H2], uh2[:, 0:H2], uh2[:, HS:H1])

                ud1 = work.tile([128, D1], bf16, tag="ud1")
                nc.vector.tensor_add(ud1[:], shdw[:, 0:D1], shdw[:, DS:H2])
                gx = fld.tile([128, FF], bf16, tag=f"gx_{name}")
                nc.vector.tensor_add(gx[:, 0:D2], ud1[:, 0:D2], ud1[:, DS:D1])
                ud2 = work.tile([128, D1], bf16, tag="ud1")
                nc.vector.tensor_add(ud2[:], dhsw[:, 0:D1], dhsw[:, DS:H2])
                gy = fld.tile([128, FF], bf16, tag=f"gy_{name}")
                nc.vector.tensor_add(gy[:, 0:D2], ud2[:, 0:D2], ud2[:, DS:D1])
                gz = fld.tile([128, FF], bf16, tag=f"gz_{name}")
                nc.vector.tensor_sub(gz[:, 0:D2], shsw[:, 2 * DS:2 * DS + D2],
                                     shsw[:, 0:D2])
                return gx, gy, gz

            def valid(tt):
                """[5,20,20] strided view (excludes h/w junk lanes)."""
                return tt[:].rearrange("p (d h w) -> p d h w",
                                       d=SD, h=BB)[:, :, 0:BC, 0:BC]

            for s in range(NSLAB):
                P = conv_fields("p", s)
                T = conv_fields("t", s)

                # |grad|^2: squares on ScalarE, adds on VectorE (flat 2x)
                def sumsq(name, G):
                    sqs = []
                    for i, g in enumerate(G):
                        sq = work.tile([128, D2], bf16, tag=f"sq{i}")
                        nc.scalar.activation(sq[:], g[:, 0:D2], AF.Square)
                        sqs.append(sq)
                    s01 = work.tile([128, D2], bf16, tag="s01")
                    nc.vector.tensor_add(s01[:], sqs[0][:], sqs[1][:])
                    ss = work.tile([128, FF], bf16, tag=f"s_{name}")
                    nc.vector.tensor_add(ss[:, 0:D2], s01[:], sqs[2][:])
                    return ss

                s_p = sumsq("p", P)
                s_t = sumsq("t", T)
                junk = work.tile([128, SD, BC, BC], bf16, tag="junko")
                nc.scalar.activation(junk[:], valid(s_p), AF.Identity,
                                     accum_out=accs[:, s:s + 1])
                nc.scalar.activation(junk[:], valid(s_t), AF.Identity,
                                     accum_out=accs[:, NSLAB + s:NSLAB + s + 1])

                # dot product (all VectorE, flat even)
                m1 = work.tile([128, D2], bf16, tag="sq0")
                m2 = work.tile([128, D2], bf16, tag="sq1")
                m3 = work.tile([128, D2], bf16, tag="sq2")
                nc.vector.tensor_mul(m1[:], P[0][:, 0:D2], T[0][:, 0:D2])
                nc.vector.tensor_mul(m2[:], P[1][:, 0:D2], T[1][:, 0:D2])
                nc.vector.tensor_mul(m3[:], P[2][:, 0:D2], T[2][:, 0:D2])
                m12 = work.tile([128, D2], bf16, tag="s01")
                nc.vector.tensor_add(m12[:], m1[:], m2[:])
                dot = work.tile([128, D2], bf16, tag="sq0")
                nc.vector.tensor_add(dot[:], m12[:], m3[:])

                # q = s_p*s_t ; ln(q) shared by sqrt(q) (mag) and rsqrt (cos)
                q = work.tile([128, FF], bf16, tag="q")
                nc.vector.tensor_mul(q[:, 0:D2], s_p[:, 0:D2], s_t[:, 0:D2])
                lnq = work.tile([128, FF], f32, tag="lnq")
                nc.scalar.activation(lnq[:, 0:D2], q[:, 0:D2], AF.Ln)
                # sum of sqrt(q) over valid voxels (mag cross-term)
                nc.scalar.activation(junk[:], valid(lnq), AF.Exp,
                                     scale=0.5,
                                     accum_out=accs[:, 2 * NSLAB + s:
                                                    2 * NSLAB + s + 1])
                r = work.tile([128, D2], bf16, tag="sq1")
                nc.scalar.activation(r[:], lnq[:, 0:D2], AF.Exp, scale=-0.5)
                c = work.tile([128, FF], bf16, tag="q")
                nc.vector.tensor_mul(c[:, 0:D2], dot[:], r[:])
                nc.scalar.activation(junk[:], valid(c), AF.Identity,
                                     accum_out=accs[:, 3 * NSLAB + s:
                                                    3 * NSLAB + s + 1])

            nc.sync.dma_start(out=out_d[:], in_=accs[:])

    nc.compile()
    return nc


def _shard_inputs(pred, target):
    """Blocked bf16 shards for the 8 cores."""
    bf = ml_dtypes.bfloat16
    starts = np.arange(0, 160, BC)  # 8 block starts per axis

    in_maps = []
    blocked = {}
    for name, x in (("pred", pred), ("targ", target)):
        per_b = []
        for b in range(2):
            gp = np.zeros((162, 162, 162), np.float32)
            gp[1:161, 1:161, 1:161] = x[b, 0]
            sw = np.lib.stride_tricks.sliding_window_view(gp, (BB, BB, BB))
            per_b.append(sw)
        blocked[name] = per_b

    for core in range(N_CORES):
        b, q = divmod(core, 4)
        m = {}
        for name in ("pred", "targ"):
            sw = blocked[name][b]
            blk = sw[np.ix_([40 * q, 40 * q + BC], starts, starts)]
            m[name] = np.ascontiguousarray(
                blk.reshape(128, FB).astype(bf))
        in_maps.append(m)
    return in_maps


def run(pred, target, trace=False):
    from concourse.bass_utils import run_bass_kernel_spmd

    pred = np.asarray(pred, dtype=np.float32)
    target = np.asarray(target, dtype=np.float32)
    assert pred.shape == (2, 1, 160, 160, 160)

    if "nc" not in _cache:
        _cache["nc"] = _build()
    nc = _cache["nc"]

    in_maps = _shard_inputs(pred, target)
    res = None
    for attempt in range(3):
        try:
            res = run_bass_kernel_spmd(
                nc, in_maps, core_ids=list(range(N_CORES)), trace=trace)
            break
        except Exception:
            if attempt == 2:
                raise
            import time as _time
            _time.sleep(5)

    sp_sum = 0.0
    st_sum = 0.0
    sq_sum = 0.0
    cos_sum = 0.0
    for core_out in res.results:
        o = np.asarray(core_out["out"], np.float64)
        sp_sum += o[:, 0:4].sum()
        st_sum += o[:, 4:8].sum()
        sq_sum += o[:, 8:12].sum()
        cos_sum += o[:, 12:16].sum()

    mag_sum = sp_sum + st_sum - 2.0 * sq_sum
    loss = WEIGHT * (mag_sum / NVOX + 1.0 - cos_sum / NVOX)
    return np.float32(loss), res.exec_time_ns


def kernel(pred, target):
    loss, _ = run(pred, target, trace=False)
    return loss



# revision 2
# speedup vs baseline: 1.0740x; 1.0740x over previous
"""Trainium2 distributed kernel for AnatomicalConsistencyLoss, v3.

Like v2 (see kernel_v2.py docstring for the engine-assignment rationale:
VectorE does all tensor_tensor work in the 2x bf16 mode, ScalarE all
unaries + accum reductions, other engines measured counterproductive),
plus:

- The w-stage (S_w / D_w) runs ONCE over the full 22-plane input volume
  instead of per-slab (saves the 2-plane slab halo reprocessing), tiled
  by DMA chunk for overlap with the input transfer.
- sw/dw and everything downstream are stored with dense 20-wide rows,
  so all h/d-stage and pointwise ops are flat 1-D contiguous slices
  (junk h-rows remain, zeroed/one'd in the field tiles and exactly
  subtracted host-side).
- The input tile and u-scratch live in a pool closed after the w-stage
  so the slab-phase tiles reuse their SBUF.
"""

import sys

import numpy as np

sys.path.insert(0, "/opt/trn_rl_repo")

import ml_dtypes

N_CORES = 8
BC = 20
BB = 22
PL = BB * BB     # 484
FB = BB ** 3     # 10648
NVOX = 2 * 160 * 160 * 160
WEIGHT = 0.2

SD = 5
NSLAB = 4
HF = SD * BC * BB    # 2200: dense field half size
NR = SD * BB         # 110 rows per half
JUNK_PER_SLAB = 10 * BC

# DMA/w-stage chunks in planes
CHUNKS = [(0, 3), (3, 8), (8, 13), (13, 17), (17, 22)]

_cache = {}


def _build():
    import concourse.bacc as bacc
    import concourse.tile as tile
    from concourse import mybir

    f32 = mybir.dt.float32
    bf16 = mybir.dt.bfloat16
    AF = mybir.ActivationFunctionType

    nc = bacc.Bacc(
        "TRN2",
        target_bir_lowering=False,
        debug=False,
        enable_asserts=False,
        num_devices=N_CORES,
    )
    xx_d = nc.dram_tensor("xx", [128, 2 * FB], bf16, kind="ExternalInput")
    out_d = nc.dram_tensor("out", [128, 48], f32, kind="ExternalOutput")

    with tile.TileContext(nc) as tc:
        with tc.tile_pool(name="pers", bufs=1) as pers:
            acc = pers.tile([128, 48], f32, tag="acc")
            sw0 = pers.tile([128, 9680], bf16, tag="sw0")
            sw1 = pers.tile([128, 9680], bf16, tag="sw1")
            dw0 = pers.tile([128, 9680], bf16, tag="dw0")
            dw1 = pers.tile([128, 9680], bf16, tag="dw1")
            swf = [sw0, sw1]
            dwf = [dw0, dw1]

            # ---- w-stage over the full volume, chunked by DMA arrival
            with tc.tile_pool(name="xp", bufs=1) as xp:
                xw = xp.tile([128, 2 * FB], bf16, tag="xw")
                U = xp.tile([128, 132 * 21], bf16, tag="u")
                for p0, p1 in CHUNKS:
                    for i in range(2):
                        nc.sync.dma_start(
                            out=xw[:, i * FB + p0 * PL:i * FB + p1 * PL],
                            in_=xx_d[:, i * FB + p0 * PL:i * FB + p1 * PL])
                for p0, p1 in CHUNKS:
                    n = (p1 - p0) * BB   # rows in chunk
                    for i in range(2):
                        xr = xw[:, i * FB + p0 * PL:i * FB + p1 * PL] \
                            .rearrange("p (r w) -> p r w", w=BB)
                        u21 = U[:, 0:n * 21].rearrange("p (r w) -> p r w",
                                                       w=21)
                        nc.vector.tensor_add(u21[:, :, :],
                                             xr[:, :, 0:21], xr[:, :, 1:22])
                        swd = swf[i][:, p0 * 440:p1 * 440] \
                            .rearrange("p (r w) -> p r w", w=BC)
                        nc.vector.tensor_add(swd[:, :, :],
                                             u21[:, :, 0:20], u21[:, :, 1:21])
                        dwd = dwf[i][:, p0 * 440:p1 * 440] \
                            .rearrange("p (r w) -> p r w", w=BC)
                        nc.vector.tensor_sub(dwd[:, :, :],
                                             xr[:, :, 2:22], xr[:, :, 0:20])

            # ---- slab phase: h/d stages + pointwise, flat dense ops
            with tc.tile_pool(name="work", bufs=1) as work, \
                 tc.tile_pool(name="fld", bufs=1) as fld, \
                 tc.tile_pool(name="late", bufs=1) as late:
                sqa = late.tile([128, 2 * HF], bf16, tag="sq0")
                sqb = late.tile([128, 2 * HF], bf16, tag="sq1")
                sqc = late.tile([128, 2 * HF], bf16, tag="sq2")
                sq = [sqa, sqb, sqc]
                stile = late.tile([128, 2 * HF], bf16, tag="s")
                M12 = late.tile([128, HF], bf16, tag="m12")
                DOT = late.tile([128, HF], bf16, tag="dot")
                Q = late.tile([128, HF], bf16, tag="q")
                RT = late.tile([128, HF], bf16, tag="r")
                SCR = late.tile([128, HF], bf16, tag="scr")
                CT = SCR   # c-mult output reuses scr (disjoint lifetimes)
                LNQ = late.tile([128, HF], f32, tag="lnq")

                def conv_hd(s, mid=None):
                    """h/d stages + squares for one slab (both inputs)."""
                    o = 2200 * s
                    ga = fld.tile([128, 2 * HF], bf16, tag="g0")
                    gb = fld.tile([128, 2 * HF], bf16, tag="g1")
                    gc = fld.tile([128, 2 * HF], bf16, tag="g2")
                    gt = [ga, gb, gc]
                    for i in range(2):
                        if i == 1 and mid is not None:
                            mid()
                        sw, dw = swf[i], dwf[i]
                        W0 = work.tile([128, 3060], bf16, tag="W0")
                        W1 = work.tile([128, 3040], bf16, tag="W1")
                        W2 = work.tile([128, 3060], bf16, tag="W2")
                        W3 = work.tile([128, 3040], bf16, tag="W3")
                        W4 = work.tile([128, 3040], bf16, tag="W4")
                        uh1, dhsw, uh2, shsw, shdw = W0, W1, W2, W3, W4
                        nc.vector.tensor_add(uh1[:, 0:3060],
                                             sw[:, o:o + 3060],
                                             sw[:, o + 20:o + 3080])
                        nc.vector.tensor_sub(dhsw[:, 0:3040],
                                             sw[:, o + 40:o + 3080],
                                             sw[:, o:o + 3040])
                        nc.vector.tensor_add(uh2[:, 0:3060],
                                             dw[:, o:o + 3060],
                                             dw[:, o + 20:o + 3080])
                        nc.vector.tensor_add(shsw[:, 0:3040],
                                             uh1[:, 0:3040], uh1[:, 20:3060])
                        nc.vector.tensor_add(shdw[:, 0:3040],
                                             uh2[:, 0:3040], uh2[:, 20:3060])
                        ud1 = work.tile([128, 2600], bf16, tag="U1")
                        ud2 = work.tile([128, 2600], bf16, tag="U2")
                        nc.vector.tensor_add(ud1[:, 0:2600],
                                             shdw[:, 0:2600],
                                             shdw[:, 440:3040])
                        nc.vector.tensor_add(ud2[:, 0:2600],
                                             dhsw[:, 0:2600],
                                             dhsw[:, 440:3040])
                        gx = gt[0][:, i * HF:(i + 1) * HF]
                        gy = gt[1][:, i * HF:(i + 1) * HF]
                        gz = gt[2][:, i * HF:(i + 1) * HF]
                        nc.vector.tensor_add(gx[:, 0:2160],
                                             ud1[:, 0:2160], ud1[:, 440:2600])
                        nc.vector.tensor_add(gy[:, 0:2160],
                                             ud2[:, 0:2160], ud2[:, 440:2600])
                        nc.vector.tensor_sub(gz[:, 0:2160],
                                             shsw[:, 880:3040],
                                             shsw[:, 0:2160])
                        for g, val in ((gx, 0.0), (gy, 0.0), (gz, 1.0)):
                            jv = g.rearrange("p (r w) -> p r w", w=440)
                            nc.vector.memset(jv[:, :, 400:440], val)
                        for k, col in ((0, 0), (1, 1), (2, 2)):
                            gh = gt[k][:, i * HF:(i + 1) * HF]
                            sh = sq[k][:, i * HF:(i + 1) * HF]
                            nc.scalar.activation(
                                sh[:, 0:HF], gh[:, 0:HF], AF.Square,
                                accum_out=acc[:, 8 * s + 2 * col + i:
                                              8 * s + 2 * col + i + 1])
                    return gt

                def pw_a_v(F, e0=0, e1=HF):
                    """V: dot products, s sums, q on flat [e0:e1)."""
                    MT = (Q, RT, SCR)
                    for m, g in zip(MT, F):
                        nc.vector.tensor_mul(m[:, e0:e1],
                                             g[:, e0:e1],
                                             g[:, HF + e0:HF + e1])
                    nc.vector.tensor_add(M12[:, e0:e1],
                                         MT[0][:, e0:e1], MT[1][:, e0:e1])
                    nc.vector.tensor_add(DOT[:, e0:e1],
                                         M12[:, e0:e1], MT[2][:, e0:e1])
                    for i in range(2):
                        h0 = i * HF
                        nc.vector.tensor_add(M12[:, e0:e1],
                                             sq[0][:, h0 + e0:h0 + e1],
                                             sq[1][:, h0 + e0:h0 + e1])
                        nc.vector.tensor_add(stile[:, h0 + e0:h0 + e1],
                                             M12[:, e0:e1],
                                             sq[2][:, h0 + e0:h0 + e1])
                    nc.vector.tensor_mul(Q[:, e0:e1],
                                         stile[:, e0:e1],
                                         stile[:, HF + e0:HF + e1])

                def pw_a_ln(e0=0, e1=HF):
                    nc.scalar.activation(LNQ[:, e0:e1], Q[:, e0:e1], AF.Ln)

                def pw_a_exp(cb, e0=0, e1=HF):
                    # r first: the downstream c-mult waits only on r
                    nc.scalar.activation(RT[:, e0:e1], LNQ[:, e0:e1],
                                         AF.Exp, scale=-0.5)
                    nc.scalar.activation(SCR[:, e0:e1], LNQ[:, e0:e1],
                                         AF.Exp, scale=0.5,
                                         accum_out=acc[:, 32 + cb:33 + cb])

                def pw_b_v(e0=0, e1=HF):
                    nc.vector.tensor_mul(CT[:, e0:e1],
                                         DOT[:, e0:e1], RT[:, e0:e1])

                def pw_b_s(cb, e0=0, e1=HF):
                    nc.scalar.activation(M12[:, e0:e1], CT[:, e0:e1],
                                         AF.Copy,
                                         accum_out=acc[:, 40 + cb:41 + cb])

                def pw_b_vred(cb, e0, e1):
                    nc.vector.tensor_mul(CT[:, e0:e1],
                                         DOT[:, e0:e1], RT[:, e0:e1])
                    nc.vector.tensor_reduce(acc[:, 40 + cb:41 + cb],
                                            CT[:, e0:e1],
                                            axis=mybir.AxisListType.X,
                                            op=mybir.AluOpType.add)

                F0 = conv_hd(0)
                pw_a_v(F0)
                pw_a_ln()
                pw_a_exp(0)
                F1 = conv_hd(1, mid=lambda: (pw_b_v(), pw_b_s(0)))
                pw_a_v(F1)
                pw_a_ln()
                pw_a_exp(1)
                F2 = conv_hd(2, mid=lambda: (pw_b_v(), pw_b_s(1)))
                pw_a_v(F2)
                pw_a_ln()
                pw_a_exp(2)
                F3 = conv_hd(3, mid=lambda: (pw_b_v(), pw_b_s(2)))
                HBE = 1100
                pw_a_v(F3, 0, HBE)
                pw_a_v(F3, HBE, HF)
                pw_a_ln(0, HBE)
                pw_a_ln(HBE, HF)
                pw_a_exp(3, 0, HBE)
                pw_b_vred(3, 0, HBE)
                pw_a_exp(4, HBE, HF)
                pw_b_vred(4, HBE, HF)

                nc.sync.dma_start(out=out_d[:], in_=acc[:])

    nc.compile()
    return nc


def _shard_inputs(pred, target):
    bf = ml_dtypes.bfloat16
    starts = np.arange(0, 160, BC)

    blocked = {}
    for name, x in (("pred", pred), ("targ", target)):
        per_b = []
        for b in range(2):
            gp = np.zeros((162, 162, 162), np.float32)
            gp[1:161, 1:161, 1:161] = x[b, 0]
            swv = np.lib.stride_tricks.sliding_window_view(gp, (BB, BB, BB))
            per_b.append(swv)
        blocked[name] = per_b

    in_maps = []
    for core in range(N_CORES):
        b, q = divmod(core, 4)
        xx = np.empty((128, 2 * FB), bf)
        for i, name in enumerate(("pred", "targ")):
            swv = blocked[name][b]
            blk = swv[np.ix_([40 * q, 40 * q + BC], starts, starts)]
            xx[:, i * FB:(i + 1) * FB] = blk.reshape(128, FB).astype(bf)
        in_maps.append({"xx": xx})
    return in_maps


def run(pred, target, trace=False):
    from concourse.bass_utils import run_bass_kernel_spmd

    pred = np.asarray(pred, dtype=np.float32)
    target = np.asarray(target, dtype=np.float32)
    assert pred.shape == (2, 1, 160, 160, 160)

    if "nc" not in _cache:
        _cache["nc"] = _build()
    nc = _cache["nc"]

    in_maps = _shard_inputs(pred, target)
    res = None
    for attempt in range(3):
        try:
            res = run_bass_kernel_spmd(
                nc, in_maps, core_ids=list(range(N_CORES)), trace=trace)
            break
        except Exception:
            if attempt == 2:
                raise
            import time as _time
            _time.sleep(5)

    sq_sum = 0.0
    sqrt_sum = 0.0
    c_sum = 0.0
    for core_out in res.results:
        o = np.asarray(core_out["out"], np.float64)
        for s in range(NSLAB):
            sq_sum += o[:, 8 * s:8 * s + 6].sum()
        sqrt_sum += o[:, 32:37].sum()
        c_sum += o[:, 40:45].sum()

    junk = float(JUNK_PER_SLAB * 128 * NSLAB * N_CORES)
    mag_sum = (sq_sum - 2 * junk) - 2.0 * (sqrt_sum - junk)
    loss = WEIGHT * (mag_sum / NVOX + 1.0 - (c_sum - junk) / NVOX)
    return np.float32(loss), res.exec_time_ns


def kernel(pred, target):
    loss, _ = run(pred, target, trace=False)
    return loss
